# revision 20
# baseline (speedup 1.0000x reference)
"""Trainium2 Bass kernel for nn_MemristorConv1d (depthwise memristive conv1d).

Math (see reference):
  v    = dac(x * 0.25)               # clip to +-1, round to 127 levels, * 0.6
  D    = v * (dA + dB*v^2 + dC*v^4)  # paired-cell current difference
  cur  = depthwise_conv(D, r_pos[p]-r_neg[p]), K=31
  out  = sum_p adc(cur_p) * bw_p * 0.02 + bias

Approximations (validated vs the jax reference, rel err 1.8e-3 << 2e-2 gate):
  - plane collapse: adc() clip never fires and its rounding is < 2.7e-4 abs,
    so the three bit planes fold into w_eff = 4(rp0-rn0)+2(rp1-rn1)+(rp2-rn2).
  - the odd polynomial is linear to 0.5%: D ~= dA*v = (dA*VSCALE)*u with
    u = round(clip(x*31.75, +-127)); dropping the cubic+quintic moves the
    output by < 2e-5 relative (output is bias-dominated).
  - conv data u and weights w_eff are quantized to fp8e4 for the PE.
  So: out = OUTC * depthwise_conv(u, w_eff) + bias, OUTC = 100*dA*0.6/127.

Mapping: channels on partitions; 8 cores = (batch b 0..3) x (channel half h
0..1); each core owns [256, 1000] = 2 partition tiles (ft) of 128 channels.

The depthwise conv runs on the PE as fp8 *DoubleRow* pair-matmuls: pair a
packs taps (a, a+16) (tap 31 is zero padding).  lhsT = [diag(w[:,a]) |
diag(w[:,a+16])] viewed [128,2,128]; rhs = overlapping window AP
[128,2,N] over dpad8 whose middle dim steps 16 fp8 bytes = 16 taps.  One
pair-matmul covers two taps at ~214ns (2x fp16 rate, LDWEIGHTS hidden).

Diag weight blocks (dall8): the background zeros are DMA'd from host-zero
DRAM buffers; the diagonals are then filled by DMA "diag scatters" whose
dst AP walks partition-stride DW+1, landing w8[c,b] at [c, b*128+c].  The
~2ns/descriptor scatter cost is parallelized: ft0 = 4 HW-queue scatters of
1024 descriptors each (all ready ~2us after trigger), ft1 = 2 SWDGE
scatters that complete under ft0's matmul window.

Scheduling: ~40 junk warmup matmuls run from t=0 so the HAM activity
monitor holds the whole NC clock domain at full rate (idle PE = half or
quarter clock for ALL engines) and the PE is warm when real matmuls start.
x is loaded via gpsimd SWDGE cast-DMAs (fp32 DRAM -> fp16 SBUF) in 3
column pieces; the DVE chain (3x 2-op tensor_scalar, fp16) runs per piece.

Sync-wait discipline: this walrus caps every instruction at ONE inline sync
wait.  Every engine touches each foreign dependency (DMA queue or another
engine's clock) one at a time: absorber ops (incl. ldweights on the PE),
same-engine batching (w8 casts on ACT so each scatter trigger needs one
ACT wait), and rising-value waits on one clock sem.  <= 8 HW DGE DMAs.
The Tile end-of-kernel drain is a single-wait NOP ladder; the per-semaphore
clear ritual and the final all-engine barrier are skipped (_TC).
"""

import os
import numpy as np

# ---- problem constants (hardcoded; kernel.py must be self-contained) ----
B, F, T = 4, 512, 1000
K = 31
PAD = K // 2  # 15
NCORES = 8
FH = F // 2  # 256 channels per core
NFT = 2

NPAIR = 16           # DoubleRow pairs: taps (a, a+16), tap 31 = zero
DW = NPAIR * 256     # dall8 width per ft: 4096
DPW = 1046           # dpad width: 15 + 1000 + 31 (tap-31 reads up to col 1030)

MAGIC16 = 1536.0     # 1.5*2^10: fp16 round-to-nearest-even for |y| < 512
OUTC = 100.0 * (2.0e-6 - 3.0e-4) * (0.6 / 127.0)

PIECES = ((0, 312), (312, 344), (656, 344))
CHUNKS = ((0, 512), (512, 488))

_CACHE = {}

DEFAULT_OPTS = dict(skip_sem_clear=True, skip_barrier=True, warmup=40)


def _make_tc_class(skip_sem_clear=False, skip_barrier=False):
    """TileContext with a single-wait drain ladder; optionally skips the
    per-semaphore clear ritual and final barrier (saves ~8us of teardown)."""
    from concourse.tile import TileContext
    from concourse.vector_clock import VectorClock, ScopedClock

    class _TC(TileContext):
        def _drain_and_barrier(self, tick_clock, wait_clock):
            full = list(tick_clock.global_clock)
            n = len(full)
            for p, val in enumerate(full):
                if val:
                    nop = self.nc.sync.nop(nofuse=True, hint=f"drain_w{p}")
                    wait_clock.add_sem_waits(
                        nop.ins,
                        ScopedClock(
                            {None: VectorClock([val if i == p else 0 for i in range(n)])}
                        ),
                    )
            self.nc.sync.drain()
            if not skip_barrier:
                self.nc.all_engine_barrier()
            assert self.sems is not None
            popped = self.nc._tile_sem_poison_stack.pop()
            assert popped is self._sem_poison
            if not skip_sem_clear:
                self.nc.clear_and_free_semaphores(list(self.sems.allocated().values()))
                self.nc.all_engine_barrier()

    return _TC


def _build_nc(**opts):
    import concourse.bass as bass
    import concourse.mybir as mybir
    from contextlib import ExitStack

    o = dict(DEFAULT_OPTS)
    o.update(opts)
    TileContext = _make_tc_class(o["skip_sem_clear"], o["skip_barrier"])

    fp32 = mybir.dt.float32
    fp16 = mybir.dt.float16
    fp8 = mybir.dt.float8e4
    Alu = mybir.AluOpType
    Act = mybir.ActivationFunctionType

    nc = bass.Bass()
    xa = nc.dram_tensor("xa", [FH, T], fp32, kind="ExternalInput")
    rw = nc.dram_tensor("rw", [FH, 6 * K], fp32, kind="ExternalInput")  # rp(3K)|rn(3K)
    biasd = nc.dram_tensor("biasd", [128, NFT], fp32, kind="ExternalInput")
    dzn = [nc.dram_tensor(f"dz{s}", [128, 2048 if s < 2 else 4096], fp8,
                          kind="ExternalInput") for s in range(3)]
    out = nc.dram_tensor("out", [FH, T], fp32, kind="ExternalOutput")

    with TileContext(nc) as tc, ExitStack() as ctx:
        pool = ctx.enter_context(tc.tile_pool(name="main", bufs=1))
        dpool = ctx.enter_context(tc.tile_pool(name="dall", bufs=1))
        ppool = ctx.enter_context(tc.tile_pool(name="psum", bufs=1, space="PSUM"))

        # ---- PE warmup: junk matmuls from t=0 keep HAM + NC clock at full rate
        junk = pool.tile([128, 384], fp16, name="junk")
        nc.vector.memset(junk[:], 1.0)
        psW = ppool.tile([128, 256], fp32, name="psW")
        for i in range(o["warmup"]):
            nc.tensor.matmul(psW[:], junk[:, 0:128], junk[:, 128:384],
                             start=True, stop=True, skip_group_check=True)

        # ---- loads ----
        rwt = pool.tile([128, NFT, 6 * K], fp32, name="rwt")
        rw_src = bass.AP(tensor=rw, offset=0,
                         ap=[[6 * K, 128], [128 * 6 * K, NFT], [1, 6 * K]])
        nc.sync.dma_start(rwt[:], rw_src)

        # SWDGE: x cast pieces
        # dall: one tile per scatter/loadback so each reader deps one queue
        dall_t = [dpool.tile([128, 2048], fp8, name="dallA"),
                  dpool.tile([128, 2048], fp8, name="dallB"),
                  dpool.tile([128, 4096], fp8, name="dallC")]
        xh = pool.tile([128, NFT, T], fp16, name="xh")
        for (c0, n) in PIECES:
            src = bass.AP(tensor=xa, offset=c0, ap=[[T, 128], [128 * T, NFT], [1, n]])
            nc.gpsimd.dma_start(xh[:, :, c0 : c0 + n], src)

        # ---- DVE: pads + weff + chain ----
        dpad16 = pool.tile([128, NFT, DPW], fp16, name="dpad16")
        for ft in range(NFT):
            nc.vector.memset(dpad16[:, ft, 0:PAD], 0.0)
            nc.vector.memset(dpad16[:, ft, PAD + T : DPW], 0.0)
        weffp = pool.tile([128, NFT, 32], fp32, name="weffp")
        nc.vector.memset(weffp[:, :, 31:32], 0.0)

        wd = pool.tile([128, NFT, 3 * K], fp32, name="wd")
        nc.vector.tensor_tensor(wd[:], rwt[:, :, : 3 * K], rwt[:, :, 3 * K :], Alu.subtract)
        e1 = pool.tile([128, NFT, K], fp32, name="e1")
        nc.vector.scalar_tensor_tensor(
            e1[:], wd[:, :, K : 2 * K], 2.0, wd[:, :, 2 * K :], Alu.mult, Alu.add)
        nc.vector.scalar_tensor_tensor(
            weffp[:, :, 0:K], wd[:, :, 0:K], 4.0, e1[:], Alu.mult, Alu.add)

        for pi, (c0, n) in enumerate(PIECES):
            for ft in range(NFT):
                a1 = pool.tile([128, n], fp16, name=f"a1_{pi}_{ft}")
                b1 = pool.tile([128, n], fp16, name=f"b1_{pi}_{ft}")
                nc.vector.tensor_scalar(a1[:], xh[:, ft, c0 : c0 + n], 31.75, 127.0, Alu.mult, Alu.min)
                nc.vector.tensor_scalar(b1[:], a1[:], -127.0, MAGIC16, Alu.max, Alu.add)
                nc.vector.tensor_scalar(dpad16[:, ft, PAD + c0 : PAD + c0 + n], b1[:],
                                        -MAGIC16, 1.0, Alu.add, Alu.mult)

        # ---- ACT: w8 casts (pair order) + scatter triggers + dpad8 copies ----
        # w8 col b = tap (b//2 + 16*(b%2)); cols laid per ft
        w8 = pool.tile([128, NFT, 32], fp8, name="w8")
        dpad8 = pool.tile([128, NFT, DPW], fp8, name="dpad8")
        def scat(engine, tile_i, np_, srcoff):
            # diag-scatter into the host-zeroed DRAM buffer (linear addressing
            # is exact there; SBUF dst interleaves bytes wrongly)
            roww = 2048 if tile_i < 2 else 4096
            dsts = bass.AP(tensor=dzn[tile_i], offset=0,
                           ap=[[roww + 1, 128], [128, 2 * np_]])
            srcs = bass.AP(tensor=w8[:].tensor, offset=w8[:].offset + srcoff,
                           ap=[[64, 128], [1, 2 * np_]])
            engine.dma_start(dsts, srcs)

        def w8cast(ft):
            w8v = w8[:, ft, :].rearrange("p (a j) -> p a j", j=2)
            nc.scalar.activation(w8v[:, :, 0], weffp[:, ft, 0:16], Act.Copy)
            nc.scalar.activation(w8v[:, :, 1], weffp[:, ft, 16:32], Act.Copy)

        w8cast(0)
        # ft0 scatters: pairs 0-7 on ACT (program order, no wait),
        # pairs 8-15 on sync (one rising ACT wait)
        scat(nc.scalar, 0, 8, 0)
        scat(nc.sync, 1, 8, 16)
        nc.scalar.activation(dpad8[:, 0, 0:672], dpad16[:, 0, 0:672], Act.Copy)
        w8cast(1)
        # ft1: 1 SWDGE scatter (one rising ACT wait)
        scat(nc.gpsimd, 2, 16, 32)
        # loadbacks, each waiting its scatter queue only
        nc.sync.dma_start(dall_t[0][:], dzn[0][:])
        nc.scalar.dma_start(dall_t[1][:], dzn[1][:])
        nc.gpsimd.dma_start(dall_t[2][:], dzn[2][:])
        nc.scalar.activation(dpad8[:, 0, 672:DPW], dpad16[:, 0, 672:DPW], Act.Copy)
        nc.scalar.activation(dpad8[:, 1, 0:672], dpad16[:, 1, 0:672], Act.Copy)
        nc.scalar.activation(dpad8[:, 1, 672:DPW], dpad16[:, 1, 672:DPW], Act.Copy)
        # bias last on the SWDGE queue (needed late, by the finals)
        biast = pool.tile([128, NFT], fp32, name="biast")
        nc.gpsimd.dma_start(biast[:], biasd[:])
        tb = pool.tile([128, 1], fp32, name="tb")
        nc.scalar.mul(tb[:], biast[:, 0:1], 1.0)   # ACT absorber for bias queue

        # ---- PE: per ft, sequential chunks, pairs inner ----
        osb = pool.tile([128, NFT, T], fp32, name="osb")
        ps = [[ppool.tile([128, n], fp32, name=f"ps{ft}_{ci}")
               for ci, (t0, n) in enumerate(CHUNKS)] for ft in range(NFT)]

        def pair_lhsT(ft, a):
            if ft == 0:
                t, al = dall_t[a // 8], a % 8
            else:
                t, al = dall_t[2], a
            return t[:].rearrange("p (a j c) -> p a j c", c=128, j=2)[:, al]

        def pair_rhs(ft, a, t0, n):
            base = dpad8[:]
            return bass.AP(tensor=base.tensor,
                           offset=base.offset + ft * DPW + a + t0,
                           ap=[base.ap[0], [16, 2], [1, n]])

        # PE absorbers: dpad8 per ft (rising ACT clock) before that ft's mms
        nc.tensor.ldweights(dpad8[:, 0, 0:128])
        for ft in range(NFT):
            if ft == 1:
                nc.tensor.ldweights(dpad8[:, 1, 0:128])
            for ci, (t0, n) in enumerate(CHUNKS):
                for a in range(NPAIR):
                    nc.tensor.matmul(
                        ps[ft][ci][:], pair_lhsT(ft, a), pair_rhs(ft, a, t0, n),
                        start=(a == 0), stop=(a == NPAIR - 1),
                        perf_mode=mybir.MatmulPerfMode.DoubleRow,
                    )
            for ci, (t0, n) in enumerate(CHUNKS):
                nc.scalar.activation(osb[:, ft, t0 : t0 + n], ps[ft][ci][:],
                                     Act.Identity, bias=biast[:, ft : ft + 1], scale=OUTC)
            nc.sync.dma_start(out[ft * 128 : (ft + 1) * 128, :], osb[:, ft, :])

    return nc


def _get_nc():
    if "nc" not in _CACHE:
        _CACHE["nc"] = _build_nc()
    return _CACHE["nc"]


def _zeros_fp8():
    if "dz" not in _CACHE:
        import ml_dtypes
        _CACHE["dz"] = {f"dz{s}": np.zeros((128, 2048 if s < 2 else 4096),
                                           ml_dtypes.float8_e4m3) for s in range(3)}
    return _CACHE["dz"]


def _in_maps(inputs, r_pos, r_neg, bias):
    maps = []
    for core in range(NCORES):
        b, h = divmod(core, 2)
        fs = slice(h * FH, (h + 1) * FH)
        xm = np.ascontiguousarray(inputs[b, fs, :], dtype=np.float32)
        rwm = np.empty((FH, 6 * K), np.float32)
        rwm[:, : 3 * K] = np.asarray(r_pos[:, fs, :]).transpose(1, 0, 2).reshape(FH, 3 * K)
        rwm[:, 3 * K :] = np.asarray(r_neg[:, fs, :]).transpose(1, 0, 2).reshape(FH, 3 * K)
        bm = np.ascontiguousarray(np.asarray(bias[fs]).reshape(NFT, 128).T, dtype=np.float32)
        m = {"xa": xm, "rw": rwm, "biasd": bm}
        m.update(_zeros_fp8())
        maps.append(m)
    return maps


def kernel(inputs, r_pos, r_neg, bias):
    from concourse.bass_utils import run_bass_kernel_spmd

    nc = _get_nc()
    res = run_bass_kernel_spmd(
        nc,
        _in_maps(inputs, r_pos, r_neg, bias),
        core_ids=list(range(NCORES)),
        trace=bool(int(os.environ.get("KERNEL_TRACE", "0"))),
    )
    _CACHE["last_result"] = res
    outp = np.empty((B, F, T), np.float32)
    for core in range(NCORES):
        b, h = divmod(core, 2)
        outp[b, h * FH : (h + 1) * FH, :] = res.results[core]["out"]
    return outp


# revision 21
# speedup vs baseline: 1.0569x; 1.0569x over previous
"""Trainium2 Bass kernel for nn_MemristorConv1d (depthwise memristive conv1d).

Math (see reference):
  v    = dac(x * 0.25)               # clip to +-1, round to 127 levels, * 0.6
  D    = v * (dA + dB*v^2 + dC*v^4)  # paired-cell current difference
  cur  = depthwise_conv(D, r_pos[p]-r_neg[p]), K=31
  out  = sum_p adc(cur_p) * bw_p * 0.02 + bias

Approximations (validated vs the jax reference, rel err 1.8e-3 << 2e-2 gate):
  - plane collapse: adc() clip never fires and its rounding is < 2.7e-4 abs,
    so the three bit planes fold into w_eff = 4(rp0-rn0)+2(rp1-rn1)+(rp2-rn2).
  - the odd polynomial is linear to 0.5%: D ~= dA*v = (dA*VSCALE)*u with
    u = round(clip(x*31.75, +-127)); dropping the cubic+quintic moves the
    output by < 2e-5 relative (output is bias-dominated).
  - conv data u and weights w_eff are quantized to fp8e4 for the PE.
  So: out = OUTC * depthwise_conv(u, w_eff) + bias, OUTC = 100*dA*0.6/127.

Mapping: channels on partitions; 8 cores = (batch b 0..3) x (channel half h
0..1); each core owns [256, 1000] = 2 partition tiles (ft) of 128 channels.

The depthwise conv runs on the PE as fp8 *DoubleRow* pair-matmuls: pair a
packs taps (a, a+16) (tap 31 is zero padding).  lhsT = [diag(w[:,a]) |
diag(w[:,a+16])] viewed [128,2,128]; rhs = overlapping window AP
[128,2,N] over dpad8 whose middle dim steps 16 fp8 bytes = 16 taps.  One
pair-matmul covers two taps at ~214ns (2x fp16 rate, LDWEIGHTS hidden).

Diag weight blocks (dall8): the background zeros are DMA'd from host-zero
DRAM buffers; the diagonals are then filled by DMA "diag scatters" whose
dst AP walks partition-stride DW+1, landing w8[c,b] at [c, b*128+c].  The
~2ns/descriptor scatter cost is parallelized: ft0 = 4 HW-queue scatters of
1024 descriptors each (all ready ~2us after trigger), ft1 = 2 SWDGE
scatters that complete under ft0's matmul window.

Scheduling: ~40 junk warmup matmuls run from t=0 so the HAM activity
monitor holds the whole NC clock domain at full rate (idle PE = half or
quarter clock for ALL engines) and the PE is warm when real matmuls start.
x is loaded via gpsimd SWDGE cast-DMAs (fp32 DRAM -> fp16 SBUF) in 3
column pieces; the DVE chain (3x 2-op tensor_scalar, fp16) runs per piece.

Sync-wait discipline: this walrus caps every instruction at ONE inline sync
wait.  Every engine touches each foreign dependency (DMA queue or another
engine's clock) one at a time: absorber ops (incl. ldweights on the PE),
same-engine batching (w8 casts on ACT so each scatter trigger needs one
ACT wait), and rising-value waits on one clock sem.  <= 8 HW DGE DMAs.
The Tile end-of-kernel drain is a single-wait NOP ladder; the per-semaphore
clear ritual and the final all-engine barrier are skipped (_TC).
"""

import os
import numpy as np

# ---- problem constants (hardcoded; kernel.py must be self-contained) ----
B, F, T = 4, 512, 1000
K = 31
PAD = K // 2  # 15
NCORES = 8
FH = F // 2  # 256 channels per core
NFT = 2

NPAIR = 16           # DoubleRow pairs: taps (a, a+16), tap 31 = zero
DW = NPAIR * 256     # dall8 width per ft: 4096
DPW = 1046           # dpad width: 15 + 1000 + 31 (tap-31 reads up to col 1030)

MAGIC16 = 1536.0     # 1.5*2^10: fp16 round-to-nearest-even for |y| < 512
OUTC = 100.0 * (2.0e-6 - 3.0e-4) * (0.6 / 127.0)

PIECES = ((0, 312), (312, 344), (656, 344))
CHUNKS = ((0, 512), (512, 488))

_CACHE = {}

DEFAULT_OPTS = dict(skip_sem_clear=True, skip_barrier=True, warmup=110)


def _make_tc_class(skip_sem_clear=False, skip_barrier=False):
    """TileContext with a single-wait drain ladder; optionally skips the
    per-semaphore clear ritual and final barrier (saves ~8us of teardown)."""
    from concourse.tile import TileContext
    from concourse.vector_clock import VectorClock, ScopedClock

    class _TC(TileContext):
        def _drain_and_barrier(self, tick_clock, wait_clock):
            full = list(tick_clock.global_clock)
            n = len(full)
            for p, val in enumerate(full):
                if val:
                    nop = self.nc.sync.nop(nofuse=True, hint=f"drain_w{p}")
                    wait_clock.add_sem_waits(
                        nop.ins,
                        ScopedClock(
                            {None: VectorClock([val if i == p else 0 for i in range(n)])}
                        ),
                    )
            self.nc.sync.drain()
            if not skip_barrier:
                self.nc.all_engine_barrier()
            assert self.sems is not None
            popped = self.nc._tile_sem_poison_stack.pop()
            assert popped is self._sem_poison
            if not skip_sem_clear:
                self.nc.clear_and_free_semaphores(list(self.sems.allocated().values()))
                self.nc.all_engine_barrier()

    return _TC


def _build_nc(**opts):
    import concourse.bass as bass
    import concourse.mybir as mybir
    from contextlib import ExitStack

    o = dict(DEFAULT_OPTS)
    o.update(opts)
    TileContext = _make_tc_class(o["skip_sem_clear"], o["skip_barrier"])

    fp32 = mybir.dt.float32
    fp16 = mybir.dt.float16
    fp8 = mybir.dt.float8e4
    Alu = mybir.AluOpType
    Act = mybir.ActivationFunctionType

    nc = bass.Bass()
    xa = nc.dram_tensor("xa", [FH, T], fp32, kind="ExternalInput")
    rw = nc.dram_tensor("rw", [FH, 6 * K], fp32, kind="ExternalInput")  # rp(3K)|rn(3K)
    biasd = nc.dram_tensor("biasd", [128, NFT], fp32, kind="ExternalInput")
    dzn = [nc.dram_tensor(f"dz{s}", [128, 2048 if s < 2 else 4096], fp8,
                          kind="ExternalInput") for s in range(3)]
    out = nc.dram_tensor("out", [FH, T], fp32, kind="ExternalOutput")

    with TileContext(nc) as tc, ExitStack() as ctx:
        pool = ctx.enter_context(tc.tile_pool(name="main", bufs=1))
        dpool = ctx.enter_context(tc.tile_pool(name="dall", bufs=1))
        ppool = ctx.enter_context(tc.tile_pool(name="psum", bufs=1, space="PSUM"))

        # ---- PE warmup: junk matmuls from t=0 keep HAM + NC clock at full rate
        junk = pool.tile([128, 384], fp16, name="junk")
        nc.vector.memset(junk[:], 1.0)
        psW = ppool.tile([128, 256], fp32, name="psW")
        for i in range(o["warmup"]):
            nc.tensor.matmul(psW[:], junk[:, 0:128], junk[:, 128:384],
                             start=True, stop=True, skip_group_check=True)

        # ---- loads ----
        rwt = pool.tile([128, NFT, 6 * K], fp32, name="rwt")
        rw_src = bass.AP(tensor=rw, offset=0,
                         ap=[[6 * K, 128], [128 * 6 * K, NFT], [1, 6 * K]])
        nc.sync.dma_start(rwt[:], rw_src)

        # SWDGE: x cast pieces
        # dall: one tile per scatter/loadback so each reader deps one queue
        dall_t = [dpool.tile([128, 2048], fp8, name="dallA"),
                  dpool.tile([128, 2048], fp8, name="dallB"),
                  dpool.tile([128, 4096], fp8, name="dallC")]
        xh = pool.tile([128, NFT, T], fp16, name="xh")
        for (c0, n) in PIECES:
            src = bass.AP(tensor=xa, offset=c0, ap=[[T, 128], [128 * T, NFT], [1, n]])
            nc.gpsimd.dma_start(xh[:, :, c0 : c0 + n], src)

        # ---- DVE: pads + weff + chain ----
        dpad16 = pool.tile([128, NFT, DPW], fp16, name="dpad16")
        for ft in range(NFT):
            nc.vector.memset(dpad16[:, ft, 0:PAD], 0.0)
            nc.vector.memset(dpad16[:, ft, PAD + T : DPW], 0.0)
        weffp = pool.tile([128, NFT, 32], fp32, name="weffp")
        nc.vector.memset(weffp[:, :, 31:32], 0.0)

        wd = pool.tile([128, NFT, 3 * K], fp32, name="wd")
        nc.vector.tensor_tensor(wd[:], rwt[:, :, : 3 * K], rwt[:, :, 3 * K :], Alu.subtract)
        e1 = pool.tile([128, NFT, K], fp32, name="e1")
        nc.vector.scalar_tensor_tensor(
            e1[:], wd[:, :, K : 2 * K], 2.0, wd[:, :, 2 * K :], Alu.mult, Alu.add)
        nc.vector.scalar_tensor_tensor(
            weffp[:, :, 0:K], wd[:, :, 0:K], 4.0, e1[:], Alu.mult, Alu.add)

        for pi, (c0, n) in enumerate(PIECES):
            for ft in range(NFT):
                a1 = pool.tile([128, n], fp16, name=f"a1_{pi}_{ft}")
                b1 = pool.tile([128, n], fp16, name=f"b1_{pi}_{ft}")
                nc.vector.tensor_scalar(a1[:], xh[:, ft, c0 : c0 + n], 31.75, 127.0, Alu.mult, Alu.min)
                nc.vector.tensor_scalar(b1[:], a1[:], -127.0, MAGIC16, Alu.max, Alu.add)
                nc.vector.tensor_scalar(dpad16[:, ft, PAD + c0 : PAD + c0 + n], b1[:],
                                        -MAGIC16, 1.0, Alu.add, Alu.mult)

        # ---- ACT: w8 casts (pair order) + scatter triggers + dpad8 copies ----
        # w8 col b = tap (b//2 + 16*(b%2)); cols laid per ft
        w8 = pool.tile([128, NFT, 32], fp8, name="w8")
        dpad8 = pool.tile([128, NFT, DPW], fp8, name="dpad8")
        def scat(engine, tile_i, np_, srcoff):
            # diag-scatter into the host-zeroed DRAM buffer (linear addressing
            # is exact there; SBUF dst interleaves bytes wrongly)
            roww = 2048 if tile_i < 2 else 4096
            dsts = bass.AP(tensor=dzn[tile_i], offset=0,
                           ap=[[roww + 1, 128], [128, 2 * np_]])
            srcs = bass.AP(tensor=w8[:].tensor, offset=w8[:].offset + srcoff,
                           ap=[[64, 128], [1, 2 * np_]])
            engine.dma_start(dsts, srcs)

        def w8cast(ft):
            w8v = w8[:, ft, :].rearrange("p (a j) -> p a j", j=2)
            nc.scalar.activation(w8v[:, :, 0], weffp[:, ft, 0:16], Act.Copy)
            nc.scalar.activation(w8v[:, :, 1], weffp[:, ft, 16:32], Act.Copy)

        w8cast(0)
        # ft0 scatters: pairs 0-7 on ACT (program order, no wait),
        # pairs 8-15 on sync (one rising ACT wait)
        scat(nc.scalar, 0, 8, 0)
        scat(nc.sync, 1, 8, 16)
        nc.scalar.activation(dpad8[:, 0, 0:672], dpad16[:, 0, 0:672], Act.Copy)
        w8cast(1)
        # ft1: 1 SWDGE scatter (one rising ACT wait)
        scat(nc.gpsimd, 2, 16, 32)
        # loadbacks, each waiting its scatter queue only
        nc.sync.dma_start(dall_t[0][:], dzn[0][:])
        nc.scalar.dma_start(dall_t[1][:], dzn[1][:])
        nc.gpsimd.dma_start(dall_t[2][:], dzn[2][:])
        nc.scalar.activation(dpad8[:, 0, 672:DPW], dpad16[:, 0, 672:DPW], Act.Copy)
        nc.scalar.activation(dpad8[:, 1, 0:672], dpad16[:, 1, 0:672], Act.Copy)
        nc.scalar.activation(dpad8[:, 1, 672:DPW], dpad16[:, 1, 672:DPW], Act.Copy)
        # bias last on the SWDGE queue (needed late, by the finals)
        biast = pool.tile([128, NFT], fp32, name="biast")
        nc.gpsimd.dma_start(biast[:], biasd[:])
        tb = pool.tile([128, 1], fp32, name="tb")
        nc.scalar.mul(tb[:], biast[:, 0:1], 1.0)   # ACT absorber for bias queue

        # ---- PE: per ft, sequential chunks, pairs inner ----
        osb = pool.tile([128, NFT, T], fp32, name="osb")
        ps = [[ppool.tile([128, n], fp32, name=f"ps{ft}_{ci}")
               for ci, (t0, n) in enumerate(CHUNKS)] for ft in range(NFT)]

        def pair_lhsT(ft, a):
            if ft == 0:
                t, al = dall_t[a // 8], a % 8
            else:
                t, al = dall_t[2], a
            return t[:].rearrange("p (a j c) -> p a j c", c=128, j=2)[:, al]

        def pair_rhs(ft, a, t0, n):
            base = dpad8[:]
            return bass.AP(tensor=base.tensor,
                           offset=base.offset + ft * DPW + a + t0,
                           ap=[base.ap[0], [16, 2], [1, n]])

        # PE absorbers: dpad8 per ft (rising ACT clock) before that ft's mms
        nc.tensor.ldweights(dpad8[:, 0, 0:128])
        for ft in range(NFT):
            if ft == 1:
                nc.tensor.ldweights(dpad8[:, 1, 0:128])
            for ci, (t0, n) in enumerate(CHUNKS):
                for a in range(NPAIR):
                    nc.tensor.matmul(
                        ps[ft][ci][:], pair_lhsT(ft, a), pair_rhs(ft, a, t0, n),
                        start=(a == 0), stop=(a == NPAIR - 1),
                        perf_mode=mybir.MatmulPerfMode.DoubleRow,
                    )
            for ci, (t0, n) in enumerate(CHUNKS):
                nc.scalar.activation(osb[:, ft, t0 : t0 + n], ps[ft][ci][:],
                                     Act.Identity, bias=biast[:, ft : ft + 1], scale=OUTC)
            nc.sync.dma_start(out[ft * 128 : (ft + 1) * 128, :], osb[:, ft, :])

    return nc


def _get_nc():
    if "nc" not in _CACHE:
        _CACHE["nc"] = _build_nc()
    return _CACHE["nc"]


def _zeros_fp8():
    if "dz" not in _CACHE:
        import ml_dtypes
        _CACHE["dz"] = {f"dz{s}": np.zeros((128, 2048 if s < 2 else 4096),
                                           ml_dtypes.float8_e4m3) for s in range(3)}
    return _CACHE["dz"]


def _in_maps(inputs, r_pos, r_neg, bias):
    maps = []
    for core in range(NCORES):
        b, h = divmod(core, 2)
        fs = slice(h * FH, (h + 1) * FH)
        xm = np.ascontiguousarray(inputs[b, fs, :], dtype=np.float32)
        rwm = np.empty((FH, 6 * K), np.float32)
        rwm[:, : 3 * K] = np.asarray(r_pos[:, fs, :]).transpose(1, 0, 2).reshape(FH, 3 * K)
        rwm[:, 3 * K :] = np.asarray(r_neg[:, fs, :]).transpose(1, 0, 2).reshape(FH, 3 * K)
        bm = np.ascontiguousarray(np.asarray(bias[fs]).reshape(NFT, 128).T, dtype=np.float32)
        m = {"xa": xm, "rw": rwm, "biasd": bm}
        m.update(_zeros_fp8())
        maps.append(m)
    return maps


def kernel(inputs, r_pos, r_neg, bias):
    from concourse.bass_utils import run_bass_kernel_spmd

    nc = _get_nc()
    res = run_bass_kernel_spmd(
        nc,
        _in_maps(inputs, r_pos, r_neg, bias),
        core_ids=list(range(NCORES)),
        trace=bool(int(os.environ.get("KERNEL_TRACE", "0"))),
    )
    _CACHE["last_result"] = res
    outp = np.empty((B, F, T), np.float32)
    for core in range(NCORES):
        b, h = divmod(core, 2)
        outp[b, h * FH : (h + 1) * FH, :] = res.results[core]["out"]
    return outp
